# revision 1
# baseline (speedup 1.0000x reference)
"""DCGRU cell Trainium2 kernel: 8-core batch-parallel (B_local=4 per core).

Diffusion (Chebyshev K=2, two supports) via dense-A blocked matmuls
(A shipped [mb, p, kb, m] bf16, streamed from HBM); gate matmuls via
DMA-transposed X^T chunks (round-tripped through DRAM) with zero-padded
per-batch W stationaries chained in PSUM; sigmoid/tanh on ACT with
per-partition bias; PE transposes fold gate outputs back to n-major.
"""
import sys
sys.path.insert(0, "/opt/trn_rl_repo")
import numpy as np
import ml_dtypes

import concourse.bass as bass
import concourse.mybir as mybir
import concourse.tile as tile
import concourse.bacc as bacc
from concourse.bass_utils import run_bass_kernel_spmd
from concourse.masks import make_identity

BF = ml_dtypes.bfloat16
bf16, f32 = mybir.dt.bfloat16, mybir.dt.float32

N, U, D = 8000, 64, 2
B, NCORES = 32, 8
F = D + U
M = 5
BL = B // NCORES
NP = 8064
NW = NP // 128
PK = BL * F
FMT = 384
OC_RU, OC_C = 2 * U, U
NWG = 512
NGRP = (NP + NWG - 1) // NWG
WPG = NWG // 128
AF = mybir.ActivationFunctionType
ALU = mybir.AluOpType


def _combos():
    out = []
    for m in range(M):
        for b_ in range(BL):
            lo, hi = b_ * F, b_ * F + F
            for ch in range(3):
                s, e = max(lo, ch * 128), min(hi, ch * 128 + 128)
                if s < e:
                    out.append((m, ch, b_, s - ch * 128, e - s, s - lo))
    return out


COMBOS = _combos()
CB = {b_: [(i, c[0], c[1]) for i, c in enumerate(COMBOS) if c[2] == b_]
      for b_ in range(BL)}
MCH = sorted({(c[0], c[1]) for c in COMBOS})


def build_program():
    nc = bacc.Bacc()
    x0h_d = nc.declare_dram_parameter("x0h", [128, NW, PK], bf16, isOutput=False)
    A_d = [
        nc.declare_dram_parameter(f"A{s}", [NW, 128, NW, 128], bf16, isOutput=False)
        for s in range(2)
    ]
    wru_d = nc.declare_dram_parameter("Wru", [len(COMBOS), 128, OC_RU], bf16, isOutput=False)
    wc_d = nc.declare_dram_parameter("Wc", [len(COMBOS), 128, OC_C], bf16, isOutput=False)
    bru_d = nc.declare_dram_parameter("bru", [OC_RU, 1], f32, isOutput=False)
    out_d = nc.declare_dram_parameter("out", [BL, NP, U], f32, isOutput=True)

    with tile.TileContext(nc) as tc:
        with (
            tc.tile_pool(name="xpool", bufs=1) as xpool,
            tc.tile_pool(name="apool", bufs=2) as apool,
            tc.tile_pool(name="wres", bufs=1) as wres,
            tc.tile_pool(name="misc", bufs=1) as misc,
            tc.tile_pool(name="xts", bufs=2) as xtsp,
            tc.tile_pool(name="sc", bufs=2) as sc,
            tc.tile_pool(name="dram", bufs=1, space="DRAM") as dram,
            tc.tile_pool(name="dram2", bufs=2, space="DRAM") as dram2,
            tc.tile_pool(name="psA", bufs=3, space="PSUM") as psA,
            tc.tile_pool(name="psW", bufs=2, space="PSUM") as psW,
            tc.tile_pool(name="psT", bufs=2, space="PSUM") as psT,
        ):
            x0 = xpool.tile([128, NW, PK], bf16, tag="x0", name="x0")
            xc = xpool.tile([128, NW, PK], bf16, tag="xc", name="xc")

            bru_t = sc.tile([OC_RU, 1], f32, tag="bru", name="bru")
            nc.sync.dma_start(bru_t[:], bru_d[:])
            ident = sc.tile([128, 128], bf16, tag="ident", name="ident")
            make_identity(nc, ident[:])

            nc.sync.dma_start(x0[:], x0h_d[:])

            HALF = NP // 2

            xm_t = [dram.tile([NP, FMT], bf16, tag=f"xm{m}", name=f"xm{m}")
                    for m in range(M)]
            zpad = sc.tile([128, NW, FMT - PK], bf16, tag="zpad", name="zpad")
            nc.vector.memset(zpad[:], 0.0)
            for m in range(M):
                nc.sync.dma_start(
                    xm_t[m][:, PK:FMT].rearrange("(w p) k -> p w k", p=128),
                    zpad[:],
                )

            def spmm(dst_tile, src_tile, s, scale2, sub_tile, dump_win):
                for mb in range(NW):
                    slab = apool.tile([128, NW, 128], bf16, tag="aslab", name="aslab")
                    nc.sync.dma_start(slab[:], A_d[s][mb])
                    ps = psA.tile([128, PK], f32, tag="ps", name="ps")
                    for kb in range(NW):
                        nc.tensor.matmul(
                            ps[:], slab[:, kb, :], src_tile[:, kb, :],
                            start=(kb == 0), stop=(kb == NW - 1),
                        )
                    if dst_tile is not None:
                        nc.scalar.activation(
                            dst_tile[:, mb, :], ps[:], AF.Copy, scale=float(scale2)
                        )
                    else:
                        stg = xtsp.tile([128, PK], bf16, tag="stg", name="stg")
                        nc.vector.tensor_tensor(
                            out=stg[:], in0=ps[:], in1=sub_tile[:, mb, :],
                            op=ALU.subtract,
                        )
                        dump_win(mb, stg)

            def gconv(w_dram, oc, sig_out):
                def dump_full(src, m):
                    nc.sync.dma_start(
                        xm_t[m][:, 0:PK].rearrange("(w p) k -> p w k", p=128),
                        src[:],
                    )

                dump_full(x0, 0)
                for s in range(2):
                    spmm(xc, x0, s, 2.0, None, None)
                    dump_full(xc, 1 + 2 * s)
                    m2 = 2 + 2 * s

                    def dw(w, stg, m2=m2):
                        nc.sync.dma_start(
                            xm_t[m2][w * 128:(w + 1) * 128, 0:PK], stg[:]
                        )
                    spmm(None, xc, s, 1.0, x0, dw)

                xt_t = dram2.tile([len(MCH), 128, NP], bf16, tag="xt_d", name="xt_d")
                for i, (m, ch) in enumerate(MCH):
                    for h in range(2):
                        xt = misc.tile([128, HALF], bf16, tag="xt", name="xt")
                        nc.sync.dma_start(
                            out=xt[:],
                            in_=xm_t[m][h * HALF:(h + 1) * HALF,
                                        ch * 128:(ch + 1) * 128],
                            transpose=True,
                        )
                        nc.sync.dma_start(
                            xt_t[i][:, h * HALF:(h + 1) * HALF], xt[:]
                        )

                wt = []
                for i in range(len(COMBOS)):
                    t = wres.tile([128, oc], bf16, tag=f"w{i}", name=f"w{i}")
                    nc.sync.dma_start(t[:], w_dram[i])
                    wt.append(t)

                for b_ in range(BL):
                    chain = CB[b_]
                    for g in range(NGRP):
                        lo = g * NWG
                        w_ = min(NWG, NP - lo)
                        pw = psW.tile([oc, NWG], f32, tag="pw", name="pw")
                        for ci, (widx, m, ch) in enumerate(chain):
                            xts = xtsp.tile([128, NWG], bf16, tag="xts", name="xts")
                            nc.sync.dma_start(
                                xts[:, :w_], xt_t[MCH.index((m, ch))][:, lo:lo + w_]
                            )
                            nc.tensor.matmul(
                                pw[:, :w_], wt[widx][:], xts[:, :w_],
                                start=(ci == 0), stop=(ci == len(chain) - 1),
                            )
                        sig_out(b_, g, lo, w_, pw)

            # ------------- gconv 1 (ru) -------------
            u_nd = dram.tile([BL, 128, NW, U], bf16, tag="u_nd", name="u_nd")

            def ru_out(b_, g, lo, w_, pw):
                rsl = xtsp.tile([U, NWG], bf16, tag="rsl", name="rsl")
                nc.scalar.activation(
                    rsl[:, :w_], pw[0:U, :w_], AF.Sigmoid, bias=bru_t[0:U, :]
                )
                usl = xtsp.tile([U, NWG], bf16, tag="usl", name="usl")
                nc.scalar.activation(
                    usl[:, :w_], pw[U:OC_RU, :w_], AF.Sigmoid, bias=bru_t[U:OC_RU, :]
                )
                for j in range(w_ // 128):
                    w = g * WPG + j
                    pt = psT.tile([128, U], bf16, tag="pt", name="pt")
                    nc.tensor.transpose(
                        pt[:], rsl[:, j * 128:(j + 1) * 128], ident[0:U, 0:U]
                    )
                    nc.vector.tensor_tensor(
                        out=x0[:, w, b_ * F + D:(b_ + 1) * F],
                        in0=pt[:],
                        in1=x0[:, w, b_ * F + D:(b_ + 1) * F],
                        op=ALU.mult,
                    )
                    ptu = psT.tile([128, U], bf16, tag="pt", name="ptu")
                    nc.tensor.transpose(
                        ptu[:], usl[:, j * 128:(j + 1) * 128], ident[0:U, 0:U]
                    )
                    ustg = xtsp.tile([128, U], bf16, tag="ustg", name="ustg")
                    nc.vector.tensor_copy(ustg[:], ptu[:])
                    nc.sync.dma_start(u_nd[b_, :, w, :], ustg[:])

            gconv(wru_d, OC_RU, ru_out)

            # ------------- gconv 2 (c) -------------
            c_nd = dram.tile([BL, 128, NW, U], bf16, tag="c_nd", name="c_nd")

            def c_out(b_, g, lo, w_, pw):
                csl = xtsp.tile([U, NWG], bf16, tag="csl", name="csl")
                nc.scalar.activation(csl[:, :w_], pw[:, :w_], AF.Tanh)
                for j in range(w_ // 128):
                    w = g * WPG + j
                    ptc = psT.tile([128, U], bf16, tag="pt", name="ptc")
                    nc.tensor.transpose(
                        ptc[:], csl[:, j * 128:(j + 1) * 128], ident[0:U, 0:U]
                    )
                    cstg = xtsp.tile([128, U], bf16, tag="ustg", name="cstg")
                    nc.vector.tensor_copy(cstg[:], ptc[:])
                    nc.sync.dma_start(c_nd[b_, :, w, :], cstg[:])

            gconv(wc_d, OC_C, c_out)

            # ------------- final combine -------------
            for b_ in range(BL):
                hxs = misc.tile([128, NW, U], bf16, tag="hxs", name="hxs")
                nc.sync.dma_start(hxs[:], x0h_d[:, :, b_ * F + D:(b_ + 1) * F])
                un = misc.tile([128, NW, U], bf16, tag="un", name="un")
                nc.sync.dma_start(un[:], u_nd[b_])
                cn = misc.tile([128, NW, U], bf16, tag="cn", name="cn")
                nc.sync.dma_start(cn[:], c_nd[b_])
                hmc = misc.tile([128, NW, U], bf16, tag="hmc", name="hmc")
                nc.vector.tensor_tensor(out=hmc[:], in0=hxs[:], in1=cn[:],
                                        op=ALU.subtract)
                um = misc.tile([128, NW, U], bf16, tag="um", name="um")
                nc.vector.tensor_tensor(out=um[:], in0=un[:], in1=hmc[:],
                                        op=ALU.mult)
                ost = misc.tile([128, NW, U], bf16, tag="ost", name="ost")
                nc.vector.tensor_tensor(out=ost[:], in0=um[:], in1=cn[:],
                                        op=ALU.add)
                nc.gpsimd.dma_start(
                    out_d[b_].rearrange("(w p) u -> p w u", p=128), ost[:]
                )

    nc.compile()
    return nc


_NC = None


def _host_prep(inputs, hx, row0, col0, val0, row1, col1, val1, W_ru, b_ru, W_c, b_c):
    inp3 = np.asarray(inputs, np.float32).reshape(B, N, D)
    hx3 = np.asarray(hx, np.float32).reshape(B, N, U)

    x0_all = np.zeros((NCORES, 128, NW, PK), BF)
    xf = np.zeros((B, NP, F), np.float32)
    xf[:, :N, :D] = inp3
    xf[:, :N, D:] = hx3
    xfw = xf.reshape(B, NW, 128, F)
    for k_ in range(NCORES):
        for b_ in range(BL):
            x0_all[k_, :, :, b_ * F:(b_ + 1) * F] = (
                xfw[k_ * BL + b_].transpose(1, 0, 2).astype(BF)
            )

    A_blocked = []
    for (r, c, v) in ((row0, col0, val0), (row1, col1, val1)):
        At = np.zeros((NP, NP), np.float32)
        np.add.at(At, (np.asarray(c), np.asarray(r)), np.asarray(v, np.float32))
        Ab = At.reshape(NW, 128, NW, 128).transpose(2, 1, 0, 3)
        A_blocked.append(np.ascontiguousarray(Ab.astype(BF)))

    def build_wzp(Wfull, oc):
        Wm = [np.asarray(Wfull, np.float32)[m::M, :].copy() for m in range(M)]
        Wm[1] *= 0.5
        Wm[3] *= 0.5
        arr = np.zeros((len(COMBOS), 128, oc), np.float32)
        for i, (m, ch, b_, flo, fcnt, foff) in enumerate(COMBOS):
            arr[i, flo:flo + fcnt, :] = Wm[m][foff:foff + fcnt, :]
        return arr.astype(BF)

    return (
        x0_all, A_blocked,
        build_wzp(W_ru, OC_RU), build_wzp(W_c, OC_C),
        np.asarray(b_ru, np.float32).reshape(OC_RU, 1),
    )


def kernel(**inputs):
    global _NC
    if _NC is None:
        _NC = build_program()
    x0_all, A_blocked, wru, wc, bru = _host_prep(**inputs)
    in_maps = [
        {"x0h": x0_all[k_], "A0": A_blocked[0], "A1": A_blocked[1],
         "Wru": wru, "Wc": wc, "bru": bru}
        for k_ in range(NCORES)
    ]
    res = run_bass_kernel_spmd(_NC, in_maps, list(range(NCORES)))
    out = np.zeros((B, N * U), np.float32)
    for k_, r in enumerate(res.results):
        o = np.asarray(r["out"], np.float32)[:, :N, :]
        out[k_ * BL:(k_ + 1) * BL] = o.reshape(BL, N * U)
    return out



# revision 6
# speedup vs baseline: 9.1535x; 9.1535x over previous
"""DCGRU cell Trainium2 kernel: 8-core batch-parallel (B_local=4 per core).

Diffusion (Chebyshev K=2, two supports) via SWDGE edge-list spmm:
dma_gather rows x[col] from DRAM -> per-edge scale on ACT (AP scale) ->
dma_scatter_add into f32 accumulator rows y[row] -> bf16 conversion pass.
Edges are host-packed into NCH chunks with each real row appearing at most
once per chunk (scatter_add RMW is not atomic across duplicate indices);
chunks are padded with dummy tokens (row 8063, val=0) so every index is
valid and the program stays static. Gate matmuls via DMA-transposed X^T
chunks with zero-padded per-batch W stationaries chained in PSUM;
sigmoid/tanh on ACT with per-partition bias; PE transposes fold gate
outputs back to n-major. Output shipped bf16 to halve wire bytes.
"""
import sys
sys.path.insert(0, "/opt/trn_rl_repo")
import numpy as np
import ml_dtypes

import concourse.bass as bass
import concourse.mybir as mybir
import concourse.tile as tile
import concourse.bacc as bacc
from concourse.bass_utils import run_bass_kernel_spmd
from concourse.library_config import mlp
from concourse.masks import make_identity

BF = ml_dtypes.bfloat16
bf16, f32, i16 = mybir.dt.bfloat16, mybir.dt.float32, mybir.dt.int16

N, U, D = 8000, 64, 2
B, NCORES = 32, 8
F = D + U
M = 5
E = 64000
BL = B // NCORES
NP = 8064
NW = NP // 128
PK = BL * F            # 264
FMT = 384              # padded gather row (bf16): 768B
SE = 320               # scatter row elems (f32): 1280B
CHUNK = 1024           # tokens per SWDGE instruction (HW limit: >1024 faults)
CBK = CHUNK // 128     # 8 blocks
CI = CHUNK // 16       # 64 idx cols
NCH = 72               # chunks per spmm (capacity 73728 >= E, >= max row degree)
DUM_R, DUM_C = NP - 1, N  # dummy scatter row / gather col (both zero rows)
OC_RU, OC_C = 2 * U, U
NWG = 512
NGRP = (NP + NWG - 1) // NWG
WPG = NWG // 128
AF = mybir.ActivationFunctionType
ALU = mybir.AluOpType


def _combos():
    out = []
    for m in range(M):
        for b_ in range(BL):
            lo, hi = b_ * F, b_ * F + F
            for ch in range(3):
                s, e = max(lo, ch * 128), min(hi, ch * 128 + 128)
                if s < e:
                    out.append((m, ch, b_, s - ch * 128, e - s, s - lo))
    return out


COMBOS = _combos()
CB = {b_: [(i, c[0], c[1]) for i, c in enumerate(COMBOS) if c[2] == b_]
      for b_ in range(BL)}
MCH = sorted({(c[0], c[1]) for c in COMBOS})


def build_program():
    nc = bacc.Bacc()
    x0h_d = nc.declare_dram_parameter("x0h", [128, NW, PK], bf16, isOutput=False)
    ci_d = [nc.declare_dram_parameter(f"ci{s}", [16, NCH * CI], i16, isOutput=False)
            for s in range(2)]
    ri_d = [nc.declare_dram_parameter(f"ri{s}", [16, NCH * CI], i16, isOutput=False)
            for s in range(2)]
    vl_d = [nc.declare_dram_parameter(f"vl{s}", [128, NCH * CBK], f32, isOutput=False)
            for s in range(2)]
    wru_d = nc.declare_dram_parameter("Wru", [len(COMBOS), 128, OC_RU], bf16, isOutput=False)
    wc_d = nc.declare_dram_parameter("Wc", [len(COMBOS), 128, OC_C], bf16, isOutput=False)
    bru_d = nc.declare_dram_parameter("bru", [OC_RU, 1], f32, isOutput=False)
    out_d = nc.declare_dram_parameter("out", [BL, NP, U], bf16, isOutput=True)

    with tile.TileContext(nc) as tc:
        with (
            tc.tile_pool(name="xpool", bufs=1) as xpool,
            tc.tile_pool(name="idxp", bufs=1) as idxp,
            tc.tile_pool(name="valp", bufs=1) as valp,
            tc.tile_pool(name="gp", bufs=2) as gp,
            tc.tile_pool(name="pp", bufs=2) as pp,
            tc.tile_pool(name="cvp", bufs=2) as cvp,
            tc.tile_pool(name="wres", bufs=1) as wres,
            tc.tile_pool(name="misc", bufs=1) as misc,
            tc.tile_pool(name="xts", bufs=2) as xtsp,
            tc.tile_pool(name="sc", bufs=2) as sc,
            tc.tile_pool(name="fin", bufs=2) as fin,
            tc.tile_pool(name="dram", bufs=1, space="DRAM") as dram,
            tc.tile_pool(name="dram2", bufs=2, space="DRAM") as dram2,
            tc.tile_pool(name="accp", bufs=2, space="DRAM") as accp,
            tc.tile_pool(name="psW", bufs=2, space="PSUM") as psW,
            tc.tile_pool(name="psT", bufs=2, space="PSUM") as psT,
        ):
            x0 = xpool.tile([128, NW, PK], bf16, tag="x0", name="x0")
            nc.sync.dma_start(x0[:], x0h_d[:])

            bru_t = sc.tile([OC_RU, 1], f32, tag="bru", name="bru")
            nc.sync.dma_start(bru_t[:], bru_d[:])
            ident = sc.tile([128, 128], bf16, tag="ident", name="ident")
            make_identity(nc, ident[:])
            # identity uses standard-library gpsimd ops (memset/affine_select);
            # switch to the mlp library (dma_gather/dma_scatter_add) only after.
            nc.gpsimd.load_library(mlp)

            HALF = NP // 2

            xm_t = [dram.tile([NP, FMT], bf16, tag=f"xm{m}", name=f"xm{m}")
                    for m in range(M)]
            zpad = sc.tile([128, NW, FMT - PK], bf16, tag="zpad", name="zpad")
            nc.vector.memset(zpad[:], 0.0)
            for m in range(M):
                nc.sync.dma_start(
                    xm_t[m][:, PK:FMT].rearrange("(w p) k -> p w k", p=128),
                    zpad[:],
                )

            # zero scratch [NP, SE] f32 for accumulator init
            zerot = sc.tile([128, SE], f32, tag="zerot", name="zerot")
            nc.vector.memset(zerot[:], 0.0)
            zscr = dram.tile([NP, SE], f32, tag="zscr", name="zscr")
            for w in range(NW):
                nc.sync.dma_start(zscr[w * 128:(w + 1) * 128, :], zerot[:])

            # per-support edge values (and x2 variants scaled by 2)
            vt, vt2 = [], []
            for s in range(2):
                v = valp.tile([128, NCH * CBK], f32, tag=f"v{s}", name=f"v{s}")
                nc.sync.dma_start(v[:], vl_d[s][:])
                v2 = valp.tile([128, NCH * CBK], f32, tag=f"v2{s}", name=f"v2{s}")
                nc.scalar.mul(v2[:], v[:], 2.0)
                vt.append(v)
                vt2.append(v2)

            colt = idxp.tile([128, NCH * CI], i16, tag="colt", name="colt")
            rowt = idxp.tile([128, NCH * CI], i16, tag="rowt", name="rowt")

            def load_idx(s):
                for r in range(8):
                    nc.sync.dma_start(colt[16 * r:16 * (r + 1), :], ci_d[s][:])
                    nc.sync.dma_start(rowt[16 * r:16 * (r + 1), :], ri_d[s][:])

            def spmm(src_m, dst_m, val_tile, sub):
                """xm_t[dst_m] = (gathered xm_t[src_m] * val) scatter-added;
                if sub: result = acc - x0 (Chebyshev x2 = 2*A*x1 - x0)."""
                acc = accp.tile([NP, SE], f32, tag="acc", name="acc")
                nc.sync.dma_start(acc[:], zscr[:])
                for c in range(NCH):
                    g = gp.tile([128, CBK, FMT], bf16, tag="g", name="g")
                    nc.gpsimd.dma_gather(
                        g[:], xm_t[src_m][:], colt[:, c * CI:(c + 1) * CI],
                        CHUNK, CHUNK, FMT,
                    )
                    pay = pp.tile([128, CBK, SE], f32, tag="pay", name="pay")
                    for blk in range(CBK):
                        j = c * CBK + blk
                        nc.scalar.activation(
                            pay[:, blk, :], g[:, blk, 0:SE], AF.Copy,
                            scale=val_tile[:, j:j + 1],
                        )
                    nc.gpsimd.dma_scatter_add(
                        acc[:], pay[:], rowt[:, c * CI:(c + 1) * CI],
                        CHUNK, CHUNK, SE,
                    )
                for w in range(NW):
                    accs = cvp.tile([128, PK], f32, tag="accs", name="accs")
                    nc.sync.dma_start(accs[:], acc[w * 128:(w + 1) * 128, 0:PK])
                    stg = cvp.tile([128, PK], bf16, tag="stg", name="stg")
                    if sub:
                        cvt = cvp.tile([128, PK], bf16, tag="cvt", name="cvt")
                        nc.vector.tensor_copy(cvt[:], accs[:])
                        nc.vector.tensor_tensor(
                            out=stg[:], in0=cvt[:], in1=x0[:, w, :],
                            op=ALU.subtract,
                        )
                    else:
                        nc.vector.tensor_copy(stg[:], accs[:])
                    nc.sync.dma_start(
                        xm_t[dst_m][w * 128:(w + 1) * 128, 0:PK], stg[:]
                    )

            def gconv(w_dram, oc, sig_out):
                nc.sync.dma_start(
                    xm_t[0][:, 0:PK].rearrange("(w p) k -> p w k", p=128),
                    x0[:],
                )
                for s in range(2):
                    load_idx(s)
                    spmm(0, 1 + 2 * s, vt[s], False)
                    spmm(1 + 2 * s, 2 + 2 * s, vt2[s], True)

                xt_t = dram2.tile([len(MCH), 128, NP], bf16, tag="xt_d", name="xt_d")
                for i, (m, ch) in enumerate(MCH):
                    for h in range(2):
                        xt = misc.tile([128, HALF], bf16, tag="xt", name="xt")
                        nc.sync.dma_start(
                            out=xt[:],
                            in_=xm_t[m][h * HALF:(h + 1) * HALF,
                                        ch * 128:(ch + 1) * 128],
                            transpose=True,
                        )
                        nc.sync.dma_start(
                            xt_t[i][:, h * HALF:(h + 1) * HALF], xt[:]
                        )

                wt = []
                for i in range(len(COMBOS)):
                    t = wres.tile([128, oc], bf16, tag=f"w{i}", name=f"w{i}")
                    nc.sync.dma_start(t[:], w_dram[i])
                    wt.append(t)

                for b_ in range(BL):
                    chain = CB[b_]
                    for g_ in range(NGRP):
                        lo = g_ * NWG
                        w_ = min(NWG, NP - lo)
                        pw = psW.tile([oc, NWG], f32, tag="pw", name="pw")
                        for ci, (widx, m, ch) in enumerate(chain):
                            xts = xtsp.tile([128, NWG], bf16, tag="xts", name="xts")
                            nc.sync.dma_start(
                                xts[:, :w_], xt_t[MCH.index((m, ch))][:, lo:lo + w_]
                            )
                            nc.tensor.matmul(
                                pw[:, :w_], wt[widx][:], xts[:, :w_],
                                start=(ci == 0), stop=(ci == len(chain) - 1),
                            )
                        sig_out(b_, g_, lo, w_, pw)

            # ------------- gconv 1 (ru) -------------
            u_nd = dram.tile([BL, 128, NW, U], bf16, tag="u_nd", name="u_nd")

            def ru_out(b_, g_, lo, w_, pw):
                rsl = xtsp.tile([U, NWG], bf16, tag="rsl", name="rsl")
                nc.scalar.activation(
                    rsl[:, :w_], pw[0:U, :w_], AF.Sigmoid, bias=bru_t[0:U, :]
                )
                usl = xtsp.tile([U, NWG], bf16, tag="usl", name="usl")
                nc.scalar.activation(
                    usl[:, :w_], pw[U:OC_RU, :w_], AF.Sigmoid, bias=bru_t[U:OC_RU, :]
                )
                for j in range(w_ // 128):
                    w = g_ * WPG + j
                    pt = psT.tile([128, U], bf16, tag="pt", name="pt")
                    nc.tensor.transpose(
                        pt[:], rsl[:, j * 128:(j + 1) * 128], ident[0:U, 0:U]
                    )
                    nc.vector.tensor_tensor(
                        out=x0[:, w, b_ * F + D:(b_ + 1) * F],
                        in0=pt[:],
                        in1=x0[:, w, b_ * F + D:(b_ + 1) * F],
                        op=ALU.mult,
                    )
                    ptu = psT.tile([128, U], bf16, tag="pt", name="ptu")
                    nc.tensor.transpose(
                        ptu[:], usl[:, j * 128:(j + 1) * 128], ident[0:U, 0:U]
                    )
                    ustg = xtsp.tile([128, U], bf16, tag="ustg", name="ustg")
                    nc.vector.tensor_copy(ustg[:], ptu[:])
                    nc.sync.dma_start(u_nd[b_, :, w, :], ustg[:])

            gconv(wru_d, OC_RU, ru_out)

            # ------------- gconv 2 (c) -------------
            c_nd = dram.tile([BL, 128, NW, U], bf16, tag="c_nd", name="c_nd")

            def c_out(b_, g_, lo, w_, pw):
                csl = xtsp.tile([U, NWG], bf16, tag="csl", name="csl")
                nc.scalar.activation(csl[:, :w_], pw[:, :w_], AF.Tanh)
                for j in range(w_ // 128):
                    w = g_ * WPG + j
                    ptc = psT.tile([128, U], bf16, tag="pt", name="ptc")
                    nc.tensor.transpose(
                        ptc[:], csl[:, j * 128:(j + 1) * 128], ident[0:U, 0:U]
                    )
                    cstg = xtsp.tile([128, U], bf16, tag="ustg", name="cstg")
                    nc.vector.tensor_copy(cstg[:], ptc[:])
                    nc.sync.dma_start(c_nd[b_, :, w, :], cstg[:])

            gconv(wc_d, OC_C, c_out)

            # ------------- final combine: new = u*hx + (1-u)*c -------------
            for b_ in range(BL):
                for w in range(NW):
                    hxs = fin.tile([128, U], bf16, tag="hxs", name="hxs")
                    nc.sync.dma_start(
                        hxs[:], x0h_d[:, w, b_ * F + D:(b_ + 1) * F]
                    )
                    un = fin.tile([128, U], bf16, tag="un", name="un")
                    nc.sync.dma_start(un[:], u_nd[b_, :, w, :])
                    cn = fin.tile([128, U], bf16, tag="cn", name="cn")
                    nc.sync.dma_start(cn[:], c_nd[b_, :, w, :])
                    hmc = fin.tile([128, U], bf16, tag="hmc", name="hmc")
                    nc.vector.tensor_tensor(out=hmc[:], in0=hxs[:], in1=cn[:],
                                            op=ALU.subtract)
                    um = fin.tile([128, U], bf16, tag="um", name="um")
                    nc.vector.tensor_tensor(out=um[:], in0=un[:], in1=hmc[:],
                                            op=ALU.mult)
                    ost = fin.tile([128, U], bf16, tag="ost", name="ost")
                    nc.vector.tensor_tensor(out=ost[:], in0=um[:], in1=cn[:],
                                            op=ALU.add)
                    nc.sync.dma_start(
                        out_d[b_, w * 128:(w + 1) * 128, :], ost[:]
                    )

    nc.compile()
    return nc


_NC = None


def _pack_edges(row, col, val):
    """Pack E edges into NCH chunks of CHUNK slots s.t. each real row index
    appears at most once per chunk (k-th edge of row r -> chunk (r+k)%NCH).
    Pad with dummy tokens (row DUM_R, col DUM_C, val 0). Returns idx/val
    arrays in SWDGE instruction layout."""
    row = np.asarray(row).astype(np.int64)
    col = np.asarray(col).astype(np.int64)
    val = np.asarray(val, np.float32)
    order = np.argsort(row, kind="stable")
    rs = row[order]
    uniq, cnt = np.unique(rs, return_counts=True)
    assert cnt.max() <= NCH, f"row degree {cnt.max()} > NCH={NCH}"
    starts = np.concatenate([[0], np.cumsum(cnt)[:-1]])
    k_sorted = np.arange(len(rs)) - np.repeat(starts, cnt)
    k = np.empty(E, np.int64)
    k[order] = k_sorted
    ch = (row + k) % NCH
    sizes = np.bincount(ch, minlength=NCH)
    assert sizes.max() <= CHUNK, f"chunk size {sizes.max()} > {CHUNK}"
    ordc = np.argsort(ch, kind="stable")
    rarr = np.full((NCH, CHUNK), DUM_R, np.int64)
    carr = np.full((NCH, CHUNK), DUM_C, np.int64)
    varr = np.zeros((NCH, CHUNK), np.float32)
    pos = 0
    for c in range(NCH):
        n = sizes[c]
        sel = ordc[pos:pos + n]
        pos += n
        rarr[c, :n] = row[sel]
        carr[c, :n] = col[sel]
        varr[c, :n] = val[sel]
    ridx = rarr.reshape(NCH, CI, 16).transpose(2, 0, 1).reshape(16, NCH * CI)
    cidx = carr.reshape(NCH, CI, 16).transpose(2, 0, 1).reshape(16, NCH * CI)
    valg = varr.reshape(NCH, CBK, 128).transpose(2, 0, 1).reshape(128, NCH * CBK)
    return (np.ascontiguousarray(ridx.astype(np.int16)),
            np.ascontiguousarray(cidx.astype(np.int16)),
            np.ascontiguousarray(valg))


def _host_prep(inputs, hx, row0, col0, val0, row1, col1, val1, W_ru, b_ru, W_c, b_c):
    inp3 = np.asarray(inputs, np.float32).reshape(B, N, D)
    hx3 = np.asarray(hx, np.float32).reshape(B, N, U)

    x0_all = np.zeros((NCORES, 128, NW, PK), BF)
    xf = np.zeros((B, NP, F), np.float32)
    xf[:, :N, :D] = inp3
    xf[:, :N, D:] = hx3
    xfw = xf.reshape(B, NW, 128, F)
    for k_ in range(NCORES):
        for b_ in range(BL):
            x0_all[k_, :, :, b_ * F:(b_ + 1) * F] = (
                xfw[k_ * BL + b_].transpose(1, 0, 2).astype(BF)
            )

    edges = [_pack_edges(row0, col0, val0), _pack_edges(row1, col1, val1)]

    def build_wzp(Wfull, oc):
        Wm = [np.asarray(Wfull, np.float32)[m::M, :].copy() for m in range(M)]
        arr = np.zeros((len(COMBOS), 128, oc), np.float32)
        for i, (m, ch, b_, flo, fcnt, foff) in enumerate(COMBOS):
            arr[i, flo:flo + fcnt, :] = Wm[m][foff:foff + fcnt, :]
        return arr.astype(BF)

    return (
        x0_all, edges,
        build_wzp(W_ru, OC_RU), build_wzp(W_c, OC_C),
        np.asarray(b_ru, np.float32).reshape(OC_RU, 1),
    )


def kernel(**inputs):
    global _NC
    if _NC is None:
        _NC = build_program()
    x0_all, edges, wru, wc, bru = _host_prep(**inputs)
    in_maps = [
        {"x0h": x0_all[k_],
         "ri0": edges[0][0], "ci0": edges[0][1], "vl0": edges[0][2],
         "ri1": edges[1][0], "ci1": edges[1][1], "vl1": edges[1][2],
         "Wru": wru, "Wc": wc, "bru": bru}
        for k_ in range(NCORES)
    ]
    res = run_bass_kernel_spmd(_NC, in_maps, list(range(NCORES)))
    out = np.zeros((B, N * U), np.float32)
    for k_, r in enumerate(res.results):
        o = np.asarray(r["out"]).astype(np.float32)[:, :N, :]
        out[k_ * BL:(k_ + 1) * BL] = o.reshape(BL, N * U)
    return out
